# revision 47
# baseline (speedup 1.0000x reference)
"""Trainium2 Bass kernel for nn_Attention_91293824844283.

Multi-head attention (identity rep): per-head 1x1-conv Q/K/V projections,
softmax(Q K^T / sqrt(E)) V, per-head output projection summed over heads.

Shapes: B=4, N=2048, D=512, H=8, E=64.

Sharding over 8 cores: core c -> (batch b = c//2, head-group g = c%2 of 4
heads). Each core computes the partial output sum over its 4 heads for its
batch; host adds the two partials per batch.

Device-side design (per core):
  - Inputs are host-packed into the exact SBUF layouts so each tensor loads
    with 1-4 large DMAs (descriptor generation on the HWDGE is serialized at
    ~630ns/DMA, so few big DMAs beat many small ones).
  - Q^T/K^T [128, N] computed per head-pair (2x64 E-rows packed on
    partitions).  V [nk, 4*66] with a ones column per head slot so the PV
    matmul also produces softmax denominators (M=65).
  - Attention per (pair, nq-quarter): both heads' S^T matmuls land in
    disjoint PE row groups and share one [128, 1024] PSUM tile so a single
    ACT exp serves the pair; PV accumulates rep~^T [65, 512] in PSUM over
    16 nk tiles.  The ACT engine is the floor (~133us of exp), so the
    emission order keeps the S^T -> exp -> PV chain first in every engine's
    priority queue.
  - All non-attention work (rep drain, normalize, output projection,
    pair-1 projections) is queued as closures and emitted one per attention
    tile, so it fills PE/DVE idle slots without ever stalling ACT.
  - Output projection packs head pairs: lhsT = [rep_h0^T; rep_h1^T] stacked
    on partitions (K=128), rhs = [Wo_h0^T; Wo_h1^T], accumulating 2 MMs
    instead of 4 per out tile.
"""

import numpy as np
import ml_dtypes
from contextlib import ExitStack

B, N, D, H, E = 4, 2048, 512, 8, 64
HPC = 4            # heads per core
N_CORES = 8
NKT = N // 128     # 16 nk tiles
VSLOT = 66         # V slot: 64 V cols + 1 ones col + 1 pad
KT = D // 128      # 4 contraction tiles for projections
QW = 512           # nq quarter width

_CACHE = {}


def _build():
    import concourse.tile as tile
    from concourse import bacc, mybir

    bf16 = mybir.dt.bfloat16
    f32 = mybir.dt.float32
    Exp = mybir.ActivationFunctionType.Exp

    nc = bacc.Bacc(
        "TRN2", target_bir_lowering=False, debug=False, num_devices=N_CORES
    )
    # x layouts (host-packed, bf16):
    #   xq: quarter-major  [128, q*2048 + k*512 + n]   (q: nq quarter)
    #   xk/xv: half-major  [128, h*4096 + k*1024 + n]  (h: nk half)
    xq = nc.dram_tensor("xq", [128, KT * N], bf16, kind="ExternalInput").ap()
    xk = nc.dram_tensor("xk", [128, KT * N], bf16, kind="ExternalInput").ap()
    xv = nc.dram_tensor("xv", [128, KT * N], bf16, kind="ExternalInput").ap()
    # wqkb: [wq_p0 | wq_p1 | wk_p0 | wk_p1 | identb]  (each w: k-major 512)
    wqkb = nc.dram_tensor("wqkb", [128, 4 * 512 + 128], bf16,
                          kind="ExternalInput").ap()
    wvb = nc.dram_tensor("wvb", [128, KT * HPC * E], bf16,
                         kind="ExternalInput").ap()
    wob = nc.dram_tensor("wob", [128, 2 * D], bf16, kind="ExternalInput").ap()
    identf = nc.dram_tensor("identf", [128, 128], f32, kind="ExternalInput").ap()
    # bf16 output halves the output DMA transfer time (~0.4% extra
    # rounding error vs a 2e-2 budget); the host converts/accumulates in f32
    outp = nc.dram_tensor("outp", [NKT, 128, D], bf16, kind="ExternalOutput").ap()

    with tile.TileContext(nc) as tc, ExitStack() as ctx:
        cp = ctx.enter_context(tc.tile_pool(name="const", bufs=1))

        # --- persistent SBUF tiles ---
        xqt = [None] + [cp.tile([128, N], bf16, tag=f"xq{q}", name=f"xq{q}")
                        for q in range(1, 4)]
        # quarter 0 is on the exp-start critical path: split it into k-half
        # tiles so the first projection matmuls start on the first DMA
        # (dependency tracking is whole-tile)
        xq0h = [cp.tile([128, 1024], bf16, tag=f"xq0h{i}", name=f"xq0h{i}")
                for i in range(2)]
        xkt = [cp.tile([128, KT * 1024], bf16, tag=f"xk{h}", name=f"xk{h}")
               for h in range(2)]
        xvt = [cp.tile([128, KT * 1024], bf16, tag=f"xv{h}", name=f"xv{h}")
               for h in range(2)]
        wqk = cp.tile([128, 4 * 512 + 128], bf16, tag="wqk", name="wqk")
        wv = cp.tile([128, KT * HPC * E], bf16, tag="wv", name="wv")
        wo = cp.tile([128, 2 * D], bf16, tag="wo", name="wo")
        idf = cp.tile([128, 128], f32, tag="idf")
        qt = [[cp.tile([128, QW], bf16, tag=f"qt{p}{c}", name=f"qt{p}{c}")
               for c in range(4)] for p in range(2)]
        kt = [cp.tile([128, N], bf16, tag=f"kt{p}", name=f"kt{p}")
              for p in range(2)]
        # vaug pair-tiles: nk-tiles 2t,2t+1 share one tile -> V-projection
        # fills both with one PSUM slot grant + one strided copy
        vaug = [cp.tile([128, 2 * HPC, VSLOT], bf16, tag=f"va{t}", name=f"va{t}")
                for t in range(NKT // 2)]
        # repb[p][t]: [128, 128] normalized rep^T chunk, heads 2p/2p+1 stacked
        repb = [[cp.tile([128, 128], bf16, tag=f"rb{p}_{t}", name=f"rb{p}_{t}")
                 for t in range(NKT)] for p in range(2)]

        def wq_s(p, k):
            off = 1024 + p * 512 + k * 128
            return wqk[:, off:off + 128]

        def wk_s(p, k):
            return wqk[:, p * 512 + k * 128:p * 512 + (k + 1) * 128]

        idb = wqk[:, 2048:2176]   # identb lives at the tail of the blob

        # --- input DMAs: few large transfers, ordered by first use.
        sy = nc.sync
        # only wk + wq_p0 (blob cols 0:1536) gate the first projections;
        # wq_p1/identb and identf load after the exp-start critical chain
        sy.dma_start(wqk[:, 0:1536], wqkb[:, 0:1536])
        sy.dma_start(xkt[0][:], xk[:, 0:4096])
        sy.dma_start(xkt[1][:], xk[:, 4096:8192])
        sy.dma_start(wv[:], wvb[:])
        sy.dma_start(xvt[0][:], xv[:, 0:4096])
        sy.dma_start(xq0h[0][:], xq[:, 0:1024])
        sy.dma_start(xq0h[1][:], xq[:, 1024:2048])
        sy.dma_start(xvt[1][:], xv[:, 4096:8192])
        sy.dma_start(xqt[1][:], xq[:, 2048:4096])
        sy.dma_start(wqk[:, 1536:2176], wqkb[:, 1536:2176])
        sy.dma_start(idf[:], identf[:])
        sy.dma_start(xqt[2][:], xq[:, 4096:6144])
        sy.dma_start(xqt[3][:], xq[:, 6144:8192])
        sy.dma_start(wo[:], wob[:])

        # --- PE warmup: dependency-free matmuls bridge the DMA window so the
        # first projection runs at full clock (HAM warm / sim p-state).
        warm_sb = cp.tile([128, 512], bf16, tag="warm_sb")
        nc.gpsimd.memset(warm_sb[:], 0.0)
        with tc.tile_pool(name="warmps", bufs=1, space="PSUM") as wps:
            wpt = wps.tile([128, 512], f32, tag="w", name="warm_ps")
            for i in range(6):
                nc.tensor.matmul(wpt[:], warm_sb[:, 0:128], warm_sb[:],
                                 start=True, stop=True)

        # --- pools.  PSUM: spair 2 banks x 2 bufs + rep 2 x 1 bank +
        # fill 2 x 1 bank = 8 banks.
        sp = ctx.enter_context(tc.tile_pool(name="spsum", bufs=2, space="PSUM"))
        rp = ctx.enter_context(tc.tile_pool(name="rpsum", bufs=1, space="PSUM"))
        fpp = ctx.enter_context(tc.tile_pool(name="fill", bufs=2, space="PSUM"))
        ptp = ctx.enter_context(tc.tile_pool(name="ptile", bufs=8))
        smp = ctx.enter_context(tc.tile_pool(name="small", bufs=4))
        rtp = ctx.enter_context(tc.tile_pool(name="rtsp", bufs=4))

        def k_chunk(p, c, tag="f"):
            # kt[p][:, c*512:(c+1)*512] += sum_k wk^T x_chunk
            ps = fpp.tile([128, QW], f32, tag=tag, name="kproj_ps")
            h, o = c // 2, (c % 2) * 512
            for k in range(KT):
                nc.tensor.matmul(
                    ps[:], wk_s(p, k), xkt[h][:, k * 1024 + o:k * 1024 + o + 512],
                    start=(k == 0), stop=(k == KT - 1),
                )
            with tc.high_priority(10**7):
                nc.vector.tensor_copy(kt[p][:, c * 512:(c + 1) * 512], ps[:])

        def q_chunk(p, c, tag="f", hot=False):
            ps = fpp.tile([128, QW], f32, tag=tag, name="qproj_ps")
            with ExitStack() as hx:
                if hot:
                    hx.enter_context(tc.high_priority(5 * 10**6))
                for k in range(KT):
                    if c == 0:
                        xs = xq0h[k // 2][:, (k % 2) * 512:(k % 2) * 512 + 512]
                    else:
                        xs = xqt[c][:, k * 512:(k + 1) * 512]
                    nc.tensor.matmul(
                        ps[:], wq_s(p, k), xs,
                        start=(k == 0), stop=(k == KT - 1),
                    )
            with tc.high_priority(10**7):
                nc.vector.tensor_copy(qt[p][c][:], ps[:])

        def v_tile(tp):
            # project nk-tiles 2tp and 2tp+1 into one psum tile
            nc.gpsimd.memset(vaug[tp][:], 1.0)
            ps = fpp.tile([128, 2 * HPC * E], f32, tag="f", name="vproj_ps")
            for j in range(2):
                t = 2 * tp + j
                h, o = t // 8, (t % 8) * 128
                for k in range(KT):
                    nc.tensor.matmul(
                        ps[:, j * 256:(j + 1) * 256],
                        xvt[h][:, k * 1024 + o:k * 1024 + o + 128],
                        wv[:, k * 256:(k + 1) * 256],
                        start=(k == 0), stop=(k == KT - 1),
                    )
            with tc.high_priority(10**7):
                nc.vector.tensor_copy(vaug[tp][:, :, 0:E], ps[:])

        # --- upfront projections for pair 0, in DMA-arrival order (the fill
        # PSUM pool grants slots in emission order, so emission must match
        # arrival or a late DMA blocks every later-emitted projection).
        k_chunk(0, 0)
        k_chunk(0, 1)
        k_chunk(0, 2)
        k_chunk(0, 3)
        for tp in range(2):
            v_tile(tp)
        # qc0 after the first V pairs (matching DMA arrival); later pairs
        # follow and finish in early-attention PE slack.  qc1 is threaded
        # between V pairs (hot: it gates quarter 1's S^T at ~tile 16).
        q_chunk(0, 0, hot=True)
        for tp in range(2, 6):
            v_tile(tp)
        q_chunk(0, 1, hot=True)
        for tp in range(6, 8):
            v_tile(tp)

        # --- deferred-work queue: closures emitted one per attention tile so
        # tail work (drain/normalize/outproj/pair-1 proj) fills engine idle
        # slots without delaying the S^T -> exp -> PV critical chain.
        pending = []
        for c in range(2, 4):
            pending.append(lambda c=c: q_chunk(0, c))

        def emit_pending(k=1):
            for _ in range(k):
                if pending:
                    pending.pop(0)()

        def make_drain(rep, rts, split=False):
            def f():
                # the drain releases the rep PSUM slot the next quarter's PV
                # needs: band it so it never queues behind normalize work.
                # split: chunk-0 columns first so the tail's first normalize
                # starts ~0.8us earlier.
                with tc.high_priority(10**7):
                    if split:
                        for s in range(2):
                            nc.vector.tensor_copy(rts[s][:, 0:128],
                                                  rep[s][:, 0:128])
                        for s in range(2):
                            nc.vector.tensor_copy(rts[s][:, 128:QW],
                                                  rep[s][:, 128:QW])
                    else:
                        for s in range(2):
                            nc.vector.tensor_copy(rts[s][:], rep[s][:])
            return f

        def make_norm_chunk(p, q4, tt, rts, rep=None, hot=False):
            def f():
                csl = slice(tt * 128, (tt + 1) * 128)
                if rep is not None:   # per-chunk drain variant (unused)
                    with tc.high_priority(10**7):
                        for s in range(2):
                            nc.vector.tensor_copy(rts[s][:, csl], rep[s][:, csl])
                rb2 = smp.tile([128, 128], bf16, tag="rb2")
                # both heads' transposes share one fill tile (fewer pool
                # grants); stage to SBUF at once so the slot frees fast.
                # In pair 1 this chain gates the output stream (and in the
                # last quarter it IS the critical path): band it.
                ctxp = tc.high_priority(10**7) if (hot or p == 1) else None
                if ctxp is not None:
                    ctxp.__enter__()
                tr12 = fpp.tile([128, 130], f32, tag="f", name="tr12")
                nc.tensor.transpose(tr12[:, 0:65], rts[0][:, csl],
                                    idf[0:65, 0:65])
                nc.tensor.transpose(tr12[:, 65:130], rts[1][:, csl],
                                    idf[0:65, 0:65])
                if hot and tt == 3:
                    t1s = tr12   # final chunk: slot-hold no longer matters
                else:
                    t1s = smp.tile([128, 130], f32, tag="t1s")
                    nc.vector.tensor_copy(t1s[:], tr12[:])
                r = smp.tile([128, 2], f32, tag="r")
                for s in range(2):
                    nc.vector.reciprocal(
                        r[:, s:s + 1], t1s[:, 64 + 65 * s:65 + 65 * s])
                    nc.vector.tensor_scalar_mul(
                        rb2[:, s * 64:(s + 1) * 64],
                        t1s[:, 65 * s:65 * s + E], r[:, s:s + 1])
                tr2 = fpp.tile([128, 128], bf16, tag="f", name="tr2")
                nc.tensor.transpose(tr2[:], rb2[:], idb)
                nc.vector.tensor_copy(repb[p][4 * q4 + tt][:], tr2[:])
                if ctxp is not None:
                    ctxp.__exit__(None, None, None)
            return f

        def make_outproj(t, tail=False):
            def f():
                if tail:
                    # last quarter: the spair PSUM slots are idle by now; a
                    # separate pool decouples outproj from the tr12/tr2 FIFO
                    ops = sp.tile([128, 2 * QW], f32, tag="s", name="ops_sp")[:, 0:D]
                else:
                    ops = fpp.tile([128, D], f32, tag="f", name="ops")
                with ExitStack() as bx:
                    if not tail:
                        bx.enter_context(tc.high_priority(10**7))
                    for p2 in range(2):
                        nc.tensor.matmul(
                            ops, repb[p2][t][:], wo[:, p2 * D:(p2 + 1) * D],
                            start=(p2 == 0), stop=(p2 == 1),
                        )
                ost = ptp.tile([128, D], bf16, tag="ost")
                if t == NKT - 1:
                    # final tile: half-granular copy+DMA shortens the tail
                    for hh in range(2):
                        hsl = slice(hh * 256, (hh + 1) * 256)
                        with tc.high_priority(10**7):
                            nc.vector.tensor_copy(ost[:, hsl], ops[:, hsl])
                        nc.sync.dma_start(outp[t][:, hsl], ost[:, hsl])
                elif tail:
                    # intermediate tail tiles: small copy pieces so a
                    # just-dispatched copy never blocks the norm chain long
                    for hh in range(4):
                        hsl = slice(hh * 128, (hh + 1) * 128)
                        nc.vector.tensor_copy(ost[:, hsl], ops[:, hsl])
                    nc.sync.dma_start(outp[t], ost[:])
                else:
                    with tc.high_priority(10**7):
                        nc.vector.tensor_copy(ost[:], ops[:])
                    nc.sync.dma_start(outp[t], ost[:])
            return f

        def attention_quarter(p, q4):
            qoff = q4 * QW
            rep = [rp.tile([65, QW], f32, tag=f"rep{s}", name=f"rep{s}")
                   for s in range(2)]
            for t in range(NKT):
                tsl = slice(t * 128, (t + 1) * 128)
                spair = sp.tile([128, 2 * QW], f32, tag="s", name="spair")
                # S^T -> exp -> PV is the critical chain (ACT is the span
                # floor): emit in dedicated priority bands so late-ready
                # projection/normalize work can never preempt it on PE.
                # S^T outranks PV: after exp(t-1) frees a PSUM slot, PE must
                # run S^T(t+1) before PV(t-1) or exp(t+1) slips ~171ns.
                with tc.high_priority(2 * 10**7):
                    for s in range(2):
                        esl = slice(s * 64, (s + 1) * 64)
                        nc.tensor.matmul(
                            spair[:, s * QW:(s + 1) * QW],
                            kt[p][esl, tsl], qt[p][q4][esl, :],
                            start=True, stop=True,
                        )
                with tc.high_priority(10**7):
                    pt = ptp.tile([128, 2 * QW], bf16, tag="p", name="pt")
                    nc.scalar.activation(pt[:], spair[:], Exp)
                    for s in range(2):
                        h = 2 * p + s
                        nc.tensor.matmul(
                            rep[s][:],
                            vaug[t // 2][:, (t % 2) * HPC + h, 0:65],
                            pt[:, s * QW:(s + 1) * QW],
                            start=(t == 0), stop=(t == NKT - 1),
                        )
                emit_pending(2)
            # queue this quarter's tail work (runs during the next quarter)
            rts = [rtp.tile([65, QW], f32, tag=f"rts{s}", name=f"rts{s}")
                   for s in range(2)]
            last = (p == 1 and q4 == 3)
            pending.append(make_drain(rep, rts, split=last))
            if p == 0:
                if q4 < 2:
                    pending.append(lambda c=2 * q4: k_chunk(1, c))
                    pending.append(lambda c=2 * q4 + 1: k_chunk(1, c))
                else:
                    pending.append(lambda c=2 * (q4 - 2): q_chunk(1, c))
                    pending.append(lambda c=2 * (q4 - 2) + 1: q_chunk(1, c))
            for tt in range(4):
                pending.append(
                    make_norm_chunk(p, q4, tt, rts, None, hot=last))
                if p == 1:
                    pending.append(make_outproj(4 * q4 + tt, tail=last))


        for p in range(2):
            for q4 in range(4):
                attention_quarter(p, q4)
        emit_pending(len(pending))

    nc.compile()
    return nc


def _prep_core_inputs(c, x1, x2, v, Wq, Wk, Wv, Wo, identf, identb):
    bf = ml_dtypes.bfloat16
    b, g = c // 2, c % 2
    hs = slice(g * HPC, (g + 1) * HPC)
    wq = (Wq[hs] * (1.0 / np.sqrt(E))).astype(np.float32)   # fold 1/sqrt(E)
    wk, wv, wo = Wk[hs], Wv[hs], Wo[hs]

    def w_pair(w, p):
        # [4,E,D] -> pair p: concat(w[2p].T, w[2p+1].T, axis=1) [D,128]
        # -> k-major [128, 512]
        m = np.concatenate([w[2 * p].T, w[2 * p + 1].T], axis=1)  # [D,128]
        return m.reshape(KT, 128, 128).transpose(1, 0, 2).reshape(128, 512)

    def x_half_major(x):
        return np.ascontiguousarray(
            x.T.reshape(KT, 128, 2, 1024).transpose(1, 2, 0, 3).reshape(128, 8192)
        ).astype(bf)

    xq = np.ascontiguousarray(
        x2[b].T.reshape(KT, 128, 4, 512).transpose(1, 2, 0, 3).reshape(128, 8192)
    ).astype(bf)
    wqkb = np.concatenate(
        [w_pair(wk, 0), w_pair(wk, 1), w_pair(wq, 0), w_pair(wq, 1),
         identb.astype(np.float32)], axis=1).astype(bf)
    wvb = np.concatenate([wv[h].T for h in range(HPC)], axis=1)  # [D, 256]
    wvb = np.ascontiguousarray(
        wvb.reshape(KT, 128, HPC * E).transpose(1, 0, 2).reshape(128, KT * HPC * E)
    ).astype(bf)
    wob = np.concatenate(
        [np.concatenate([wo[2 * p].T, wo[2 * p + 1].T], axis=0)  # [128, D]
         for p in range(2)], axis=1).astype(bf)                  # [128, 2D]
    return {
        "xq": xq, "xk": x_half_major(x1[b]), "xv": x_half_major(v[b]),
        "wqkb": wqkb, "wvb": wvb, "wob": wob,
        "identf": identf,
    }


def kernel(**inputs):
    from concourse.bass_utils import run_bass_kernel_spmd

    x1 = np.asarray(inputs["x1"], np.float32)
    x2 = np.asarray(inputs["x2"], np.float32)
    v = np.asarray(inputs["v"], np.float32)
    Wq = np.asarray(inputs["Wq"], np.float32)
    Wk = np.asarray(inputs["Wk"], np.float32)
    Wv = np.asarray(inputs["Wv"], np.float32)
    Wo = np.asarray(inputs["Wo"], np.float32)

    if "nc" not in _CACHE:
        _CACHE["nc"] = _build()
    nc = _CACHE["nc"]

    identf = np.eye(128, dtype=np.float32)
    identb = np.eye(128, dtype=ml_dtypes.bfloat16)
    in_maps = [
        _prep_core_inputs(c, x1, x2, v, Wq, Wk, Wv, Wo, identf, identb)
        for c in range(N_CORES)
    ]
    res = run_bass_kernel_spmd(nc, in_maps, list(range(N_CORES)))
    out = np.empty((B, N, D), np.float32)
    for b in range(B):
        out[b] = (
            res.results[2 * b]["outp"].reshape(N, D).astype(np.float32)
            + res.results[2 * b + 1]["outp"].reshape(N, D).astype(np.float32)
        )
    return out
